# revision 67
# baseline (speedup 1.0000x reference)
"""Trainium2 Bass kernel for the NNConv/GNN message-passing problem.

Strategy (graph-parallel over 8 cores, 128 graphs each):
  * Edge features take only 8^3=512 distinct values -> the edge-conditioned
    weight MLP (99% of reference FLOPs) is deduplicated into a 512-entry
    table of [64,32] matrices, computed host-side (parameter-only
    transform) and shipped as part of one bf16 weight blob.
  * The replicated weight blob (atom table + W-table + readout weights,
    ~3.1 MB) is uploaded SHARDED 1/8th per core and AllGathered on-device
    over NeuronLink, cutting per-call host->device transfer 8x. (Baking it
    into the NEFF as Const tensors was tried and is SLOWER: the base64
    const data rides in the custom-call backend_config and costs ~134
    ms/MB in per-call jit lowering, vs ~29 ms/MB for the AllGather.)
  * Gather index tensors are uploaded unreplicated [16, n/16] and
    replicated to the 8 gpsimd cores' partition groups on device.
  * The segment-sum one-hot (dhat) is built on device from compact int16
    dst indices with a single broadcast is_equal against an iota row.
    NOTE: dma_gather with num_idxs > 512 crashes this runtime -- CH=512
    is a hard limit (isolated across two crash/bisect cycles).
  * Node encoder: dma_gather over the stacked [1152,128]-padded bf16
    embedding table + vector adds.
  * Messages: edges type-sorted (host-computed permutation); per-type
    matmul  msg[n_t,32] = XS_T[64,n_t].T @ Wtab[t].
  * segment_sum + root + bias: per-graph one-hot matmul
    aggT[32,40] = msg_g[128,32].T @ Dhat_g[128,40], accumulated into a
    transposed feature plane F[32,5120] initialized with x@root + bias.
  * Readout MLP runs transposed (features on partitions, graphs on free),
    biases applied per-partition by the scalar engine.
"""

import numpy as np
import ml_dtypes

import concourse.bass as bass
import concourse.bacc as bacc
import concourse.mybir as mybir
import concourse.tile as tile
from concourse import library_config
from concourse.bass_utils import run_bass_kernel_spmd

BF16 = ml_dtypes.bfloat16
F32 = np.float32

G, NPG, EPG, MAXN = 1024, 40, 80, 51
D_IN, D_OUT, D_EDGE = 64, 32, 16
NCORES = 8
GPC = G // NCORES          # 128 graphs / core
NPC = GPC * NPG            # 5120 nodes / core
EPC = GPC * EPG            # 10240 edges / core
NTYPES = 512
VOC = 9 * 128              # stacked atom-embedding rows

# ---- weight blobs: blob1 bf16 (atab + small readout tensors), blob2 fp8
# (wtab + w1 + w2, each globally scaled; de-scaled on device). Two cc
# stages so the big fp8 AllGather overlaps encoder work ----
ATAB_N = VOC * 128                 # 147456
WTAB_N = 64 * NTYPES * 32          # 1048576
WROS_COLS = 32 + 32 + 8 + 1        # w3,rootp,w4,w5 = 73
WROS_N = 128 * WROS_COLS           # 9344
W1_N = 128 * 2560                  # 327680
W2_N = 128 * 256                   # 32768
B1_N = ATAB_N + WROS_N             # 156800 (divisible by 8)
B2_N = WTAB_N + W1_N + W2_N        # 1409024 (divisible by 8)
S1_N = B1_N // NCORES              # 19600
S2_N = B2_N // NCORES              # 176128

# ---- index blob layout ([16, n/16] int16 columns) ----
ENC_W = 9 * NPC // 16              # 2880
RG_W = GPC * 128 // 16             # 1024


def _wrap16(idx):
    """int16 index array -> [16, n/16] (16-partition wrap, unreplicated)."""
    idx = np.asarray(idx, np.int16)
    n = idx.shape[0]
    assert n % 16 == 0
    w = np.empty((16, n // 16), np.int16)
    for p in range(16):
        w[p, :] = idx[p::16]
    return w


def _build_program(C, inv_s, use_cc=True):
    """Emit the SPMD Tile program. C = per-type capacity (multiple of 64).

    inv_s: de-scale factor for the fp8-shipped W-table.
    use_cc=True: weight blobs arrive sharded 1/8 per core and are
    AllGathered on device. use_cc=False (CoreSim): full blob inputs.
    """
    dt = mybir.dt
    nc = bacc.Bacc("TRN2", target_bir_lowering=False, debug=False)

    NXS = NTYPES * C           # type-padded edge columns
    NXT = NXS + NPC            # + identity (x.T) columns
    assert C % 64 == 0
    CHUNKS = NXS // 128        # msg psum chunks of 128 rows
    XT_W = NXT // 16
    IDX_W = ENC_W + XT_W + RG_W

    # ---- DRAM I/O ----
    if use_cc:
        wshard = nc.dram_tensor("wshard", [S1_N], dt.bfloat16,
                                kind="ExternalInput")
        wshard8 = nc.dram_tensor("wshard8", [S2_N], dt.float8e4,
                                 kind="ExternalInput")
        wcc1 = nc.dram_tensor("wcc1", [S1_N], dt.bfloat16)
        wcc2 = nc.dram_tensor("wcc2", [S2_N], dt.float8e4)
        wfull1 = nc.dram_tensor("wfull1", [B1_N], dt.bfloat16)
        wfull2 = nc.dram_tensor("wfull2", [B2_N], dt.float8e4)
    else:
        wfull1 = nc.dram_tensor("wshard", [B1_N], dt.bfloat16,
                                kind="ExternalInput")
        wfull2 = nc.dram_tensor("wshard8", [B2_N], dt.float8e4,
                                kind="ExternalInput")
    fb = nc.dram_tensor("fb", [128, 7], dt.float32, kind="ExternalInput")
    idx16 = nc.dram_tensor("idx16", [16, IDX_W], dt.int16,
                           kind="ExternalInput")
    dstc = nc.dram_tensor("dstc", [128, GPC], dt.int16, kind="ExternalInput")

    x_dram = nc.dram_tensor("x_scr", [NPC + 128, 128], dt.bfloat16)
    msg_dram = nc.dram_tensor("msg_scr", [NXS, 64], dt.float32)
    y = nc.dram_tensor("y", [1, GPC], dt.float32, kind="ExternalOutput")

    with tile.TileContext(nc) as tc:
        ch_reg = [None]

        def chunked_gather(dst3, srcT, idx, total, elem, transpose=False):
            CH = 512
            assert total % CH == 0
            if ch_reg[0] is None:
                ch_reg[0] = nc.gpsimd.to_reg(CH)
            for k in range(total // CH):
                isl = idx[:, k * (CH // 16):(k + 1) * (CH // 16)]
                if transpose:
                    osl = dst3[:, :, k * CH:(k + 1) * CH]
                else:
                    osl = dst3[:, k * (CH // 128):(k + 1) * (CH // 128), :]
                nc.gpsimd.dma_gather(osl, srcT, isl, CH, ch_reg[0], elem,
                                     transpose=transpose)

        nc.gpsimd.load_library(library_config.mlp)

        # ---- AllGather the weight blob (sharded upload, 2 stages so the
        # big wtab transfer overlaps encoder work that needs only blob1) ----
        if use_cc:
            nc.gpsimd.dma_start(wcc1[:], wshard[:])
            nc.gpsimd.collective_compute(
                "AllGather", mybir.AluOpType.bypass,
                replica_groups=[list(range(NCORES))],
                ins=[wcc1[:]], outs=[wfull1[:]])
            nc.gpsimd.dma_start(wcc2[:], wshard8[:])
            nc.gpsimd.collective_compute(
                "AllGather", mybir.AluOpType.bypass,
                replica_groups=[list(range(NCORES))],
                ins=[wcc2[:]], outs=[wfull2[:]])
        atab_v = wfull1[0:ATAB_N].rearrange("(v d) -> v d", v=VOC)
        wro_v = wfull1[ATAB_N:B1_N].rearrange("(p f) -> p f", p=128)
        wtab_v = wfull2[0:WTAB_N].rearrange("(p f) -> p f", p=64)
        w1_v = wfull2[WTAB_N:WTAB_N + W1_N].rearrange("(p f) -> p f", p=128)
        w2_v = wfull2[WTAB_N + W1_N:B2_N].rearrange("(p f) -> p f", p=128)

        inv_sw, inv_s1, inv_s2 = inv_s
        with tc.tile_pool(name="persist", bufs=1) as pp:
            # ---- persistent tiles (wtab/w1/w2 ship fp8-e4m3, rescaled) ----
            wtab8 = pp.tile([64, NTYPES * 32], dt.float8e4)
            nc.sync.dma_start(wtab8[:], wtab_v)
            wtab_t = pp.tile([64, NTYPES * 32], dt.bfloat16)
            nc.vector.tensor_scalar(
                wtab_t[:], wtab8[:], float(inv_sw), None,
                op0=mybir.AluOpType.mult)
            wtab = wtab_t[:].rearrange("p (t o) -> p t o", o=32)
            w18 = pp.tile([128, 2560], dt.float8e4)
            nc.sync.dma_start(w18[:], w1_v)
            w1c = pp.tile([128, 2560], dt.bfloat16)
            nc.vector.tensor_scalar(w1c[:], w18[:], float(inv_s1), None,
                                    op0=mybir.AluOpType.mult)
            w28 = pp.tile([128, 256], dt.float8e4)
            nc.sync.dma_start(w28[:], w2_v)
            w2c = pp.tile([128, 256], dt.bfloat16)
            nc.vector.tensor_scalar(w2c[:], w28[:], float(inv_s2), None,
                                    op0=mybir.AluOpType.mult)
            wro = pp.tile([128, WROS_COLS], dt.bfloat16)
            nc.sync.dma_start(wro[:], wro_v)
            fb_sb = pp.tile([128, 7], dt.float32)
            nc.sync.dma_start(fb_sb[:], fb[:])
            idx_sb = pp.tile([128, IDX_W], dt.int16)
            for k in range(8):
                nc.sync.dma_start(idx_sb[16 * k:16 * (k + 1), :], idx16[:])
            dstc_sb = pp.tile([128, GPC], dt.int16)
            nc.sync.dma_start(dstc_sb[:], dstc[:])
            F = pp.tile([32, NPC], dt.float32)

            rootp_sb = wro[:, 32:64]
            cbias_sb = fb_sb[0:32, 0:1]

            # ---- dhat one-hot from dst indices (exact 0/1 in fp32) ----
            iota_m = pp.tile([128, 40], dt.int16)
            nc.gpsimd.iota(iota_m[:], pattern=[[1, 40]], channel_multiplier=0)
            dhat_sb = pp.tile([128, GPC, 40], dt.float32)
            nc.vector.tensor_tensor(
                dhat_sb[:],
                dstc_sb[:].rearrange("p (g o) -> p g o", o=1)
                .to_broadcast([128, GPC, 40]),
                iota_m[:].rearrange("p (o m) -> p o m", o=1)
                .to_broadcast([128, GPC, 40]),
                op=mybir.AluOpType.is_equal)

            # ---- encoder: gather 9 embedding rows/node in 3 passes, sum ----
            with tc.tile_pool(name="enc", bufs=2) as ep:
                eidx = idx_sb[:, 0:ENC_W]
                S = NPC // 128  # 40 slots per feature column
                x_bf = ep.tile([128, S, 128], dt.bfloat16)
                NB = 3 * NPC
                for b in range(3):
                    epart = ep.tile([128, NB // 128, 128], dt.bfloat16,
                                    tag="epart")
                    chunked_gather(
                        epart[:], atab_v,
                        eidx[:, b * (NB // 16):(b + 1) * (NB // 16)],
                        NB, 128)
                    if b == 0:
                        nc.vector.tensor_tensor(
                            x_bf[:], epart[:, 0:S, :], epart[:, S:2 * S, :],
                            op=mybir.AluOpType.add)
                        nc.vector.tensor_tensor(
                            x_bf[:], x_bf[:], epart[:, 2 * S:3 * S, :],
                            op=mybir.AluOpType.add)
                    else:
                        for j in range(3):
                            nc.vector.tensor_tensor(
                                x_bf[:], x_bf[:], epart[:, j * S:(j + 1) * S, :],
                                op=mybir.AluOpType.add)
                # stage x rows (+ one zero block) to DRAM for the src-gather
                xv = x_dram.ap().rearrange("(s p) d -> p s d", p=128)
                nc.sync.dma_start(xv[:, 0:S, :], x_bf[:])
                zrow = ep.tile([128, 1, 128], dt.bfloat16)
                nc.vector.memset(zrow[:], 0.0)
                nc.sync.dma_start(xv[:, S:S + 1, :], zrow[:])

            # ---- transposed gather: XS_T (type-sorted) ++ x.T ----
            xtp_cm = tc.tile_pool(name="xtp", bufs=1)
            xp = xtp_cm.__enter__()
            xt = xp.tile([128, 1, NXT], dt.bfloat16)
            xidx = idx_sb[:, ENC_W:ENC_W + XT_W]
            chunked_gather(xt[:], x_dram[:], xidx, NXT, 128, transpose=True)
            xtv = xt[:, 0, :]

            # ---- F init: x @ root + conv_bias (transposed) ----
            psp_cm = tc.tile_pool(name="ps_mid", bufs=3, space="PSUM")
            psp = psp_cm.__enter__()
            for nch in range(NPC // 512):
                ps = psp.tile([32, 512], dt.float32, tag="xr")
                nc.tensor.matmul(ps[:], rootp_sb,
                                 xtv[:, NXS + nch * 512: NXS + (nch + 1) * 512],
                                 start=True, stop=True)
                nc.scalar.activation(F[:, nch * 512:(nch + 1) * 512], ps[:],
                                     mybir.ActivationFunctionType.Identity,
                                     bias=cbias_sb)

            # ---- messages: per-type matmuls, 16 chunks per PSUM bank ----
            with tc.tile_pool(name="msgp", bufs=6) as mp:
                msgv = msg_dram.ap().rearrange("(s p) d -> p s d", p=128)
                for g16 in range(CHUNKS // 16):
                    ps = psp.tile([128, 16, 32], dt.float32, tag="msg")
                    for j in range(16):
                        ch = g16 * 16 + j
                        for half in range(128 // 64):
                            col = ch * 128 + half * 64
                            nc.tensor.matmul(ps[half * 64:(half + 1) * 64, j, :],
                                             xtv[0:64, col:col + 64],
                                             wtab[:, col // C, :],
                                             start=True, stop=True)
                    st = mp.tile([128, 16, 32], dt.float32, tag="stage")
                    nc.vector.tensor_copy(st[:], ps[:])
                    nc.sync.dma_start(msgv[:, g16 * 16:(g16 + 1) * 16, 0:32],
                                      st[:])
            psp_cm.__exit__(None, None, None)
            xtp_cm.__exit__(None, None, None)

            # ---- regather per graph (128 rows each) + scatter matmul ----
            with tc.tile_pool(name="scat", bufs=1) as sp:
                ridx = idx_sb[:, ENC_W + XT_W:IDX_W]
                gt = sp.tile([128, GPC, 64], dt.float32)
                chunked_gather(gt[:], msg_dram[:], ridx, GPC * 128, 64)
                psp_cm = tc.tile_pool(name="ps_sc", bufs=6, space="PSUM")
                psp = psp_cm.__enter__()
                NG = 12
                for g0 in range(0, GPC, NG):
                    n = min(NG, GPC - g0)
                    ps = psp.tile([32, NG * 40], dt.float32, tag="sc")
                    for j in range(n):
                        g = g0 + j
                        nc.tensor.matmul(ps[:, j * 40:(j + 1) * 40],
                                         gt[:, g, 0:32], dhat_sb[:, g, :],
                                         start=True, stop=True)
                    nc.vector.tensor_tensor(
                        F[:, g0 * 40:(g0 + n) * 40],
                        F[:, g0 * 40:(g0 + n) * 40], ps[:, 0:n * 40],
                        op=mybir.AluOpType.add)
                psp_cm.__exit__(None, None, None)

            # ---- fold F[32,5120] -> F2[128,1280] (bf16) ----
            with tc.tile_pool(name="ro", bufs=1) as rp:
                F2 = rp.tile([128, GPC * 10], dt.bfloat16)
                Fv = F[:].rearrange("p (g q j) -> p g q j", g=GPC, q=10)
                for j in range(4):
                    dst = F2[j * 32:(j + 1) * 32, :].rearrange(
                        "p (g q) -> p g q", g=GPC)
                    nc.vector.tensor_copy(dst, Fv[:, :, :, j])

                # ---- readout MLP (transposed, biases per-partition) ----
                w1_sb = w1c[:].rearrange("p (q r) -> p q r", q=10)
                w2_sb = w2c[:].rearrange("p (h r) -> p h r", h=2)
                w3_sb = wro[:, 0:32]
                w4_sb = wro[0:32, 64:72]
                w5_sb = wro[0:8, 72:73]
                mb1_sb = fb_sb[:, 1:3]
                mb2_sb = fb_sb[:, 3:4]
                mb3_sb = fb_sb[0:32, 4:5]
                mb4_sb = fb_sb[0:8, 5:6]
                mb5_sb = fb_sb[0:1, 6:7]

                psp_cm = tc.tile_pool(name="ps_ro", bufs=2, space="PSUM")
                psp = psp_cm.__enter__()
                F2q = F2[:].rearrange("p (g q) -> p q g", q=10)
                a1 = rp.tile([128, 2, GPC], dt.bfloat16)
                for mh in range(2):
                    ps = psp.tile([128, GPC], dt.float32, tag="ro1")
                    for q in range(10):
                        nc.tensor.matmul(ps[:], w1_sb[:, q, mh * 128:(mh + 1) * 128],
                                         F2q[:, q, :], start=(q == 0), stop=(q == 9))
                    nc.scalar.activation(a1[:, mh, :], ps[:],
                                         mybir.ActivationFunctionType.Relu,
                                         bias=mb1_sb[:, mh:mh + 1])
                ps2 = psp.tile([128, GPC], dt.float32, tag="ro1")
                for h in range(2):
                    nc.tensor.matmul(ps2[:], w2_sb[:, h, :], a1[:, h, :],
                                     start=(h == 0), stop=(h == 1))
                a2 = rp.tile([128, GPC], dt.bfloat16)
                nc.scalar.activation(a2[:], ps2[:],
                                     mybir.ActivationFunctionType.Relu,
                                     bias=mb2_sb)
                ps3 = psp.tile([32, GPC], dt.float32, tag="ro2")
                nc.tensor.matmul(ps3[:], w3_sb, a2[:], start=True, stop=True)
                a3 = rp.tile([32, GPC], dt.bfloat16)
                nc.scalar.activation(a3[:], ps3[:],
                                     mybir.ActivationFunctionType.Relu,
                                     bias=mb3_sb)
                ps4 = psp.tile([8, GPC], dt.float32, tag="ro2")
                nc.tensor.matmul(ps4[:], w4_sb, a3[:], start=True, stop=True)
                a4 = rp.tile([8, GPC], dt.bfloat16)
                nc.scalar.activation(a4[:], ps4[:],
                                     mybir.ActivationFunctionType.Relu,
                                     bias=mb4_sb)
                ps5 = psp.tile([1, GPC], dt.float32, tag="ro2")
                nc.tensor.matmul(ps5[:], w5_sb, a4[:], start=True, stop=True)
                yv = rp.tile([1, GPC], dt.float32)
                nc.scalar.activation(yv[:], ps5[:],
                                     mybir.ActivationFunctionType.Identity,
                                     bias=mb5_sb)
                nc.sync.dma_start(y[:], yv[:])
                psp_cm.__exit__(None, None, None)

    nc.compile()
    return nc


def _host_prep(node_features, edge_features, edge_index, batch,
               atom_emb, bond_emb, gW1, gW2, gW3, root, conv_bias, mws, mbs,
               use_cc=True):
    """Build per-core input maps + pick type capacity C."""
    nf = np.asarray(node_features, np.int64)
    ef = np.asarray(edge_features, np.int64)
    src = np.asarray(edge_index, np.int64)[0]
    dst = np.asarray(edge_index, np.int64)[1]
    atom_emb = np.asarray(atom_emb, F32)
    bond_emb = np.asarray(bond_emb, F32)
    gW1 = np.asarray(gW1, F32); gW2 = np.asarray(gW2, F32); gW3 = np.asarray(gW3, F32)
    root = np.asarray(root, F32); conv_bias = np.asarray(conv_bias, F32)
    mws = [np.asarray(w, F32) for w in mws]
    mbs = [np.asarray(b, F32) for b in mbs]

    # ---- replicated weight blob ----
    atab = np.zeros((VOC, 128), BF16)
    atab[:, :64] = atom_emb.reshape(VOC, 64).astype(BF16)

    # host-computed 512-entry weight table (parameter-only transform)
    tt = np.arange(NTYPES)
    i0, i1, i2 = tt // 64, (tt // 8) % 8, tt % 8
    E = bond_emb[0, i0] + bond_emb[1, i1] + bond_emb[2, i2]   # [512,16]
    h = np.maximum(E @ gW1, 0.0)
    h = np.maximum(h @ gW2, 0.0)
    W = (h @ gW3).reshape(NTYPES, 64, 32)                     # [t,d,o]
    wtab_f32 = W.transpose(1, 0, 2).reshape(64, NTYPES * 32)

    def _q8(a):
        s = 120.0 / max(1e-30, float(np.abs(a).max()))
        return (a * s).astype(ml_dtypes.float8_e4m3), 1.0 / s

    rootp = np.zeros((128, 32), F32)
    rootp[:64] = root
    # readout weights: w1 reordered [(j*32+oo), q, r] = mW1[(4q+j)*32+oo, r]
    w1r = mws[0][:1280].reshape(40, 32, 256).reshape(10, 4, 32, 256) \
        .transpose(1, 2, 0, 3).reshape(128, 2560)
    w2r = mws[1].reshape(2, 128, 128).transpose(1, 0, 2).reshape(128, 256)
    wro = np.zeros((128, WROS_COLS), F32)
    wro[:, 0:32] = mws[2]                 # w3 [128,32]
    wro[:, 32:64] = rootp
    wro[0:32, 64:72] = mws[3]             # w4 [32,8]
    wro[0:8, 72:73] = mws[4]              # w5 [8,1]
    wtab8, inv_sw = _q8(wtab_f32)
    w18, inv_s1 = _q8(w1r)
    w28, inv_s2 = _q8(w2r)
    inv_s = (inv_sw, inv_s1, inv_s2)
    blob1 = np.concatenate([atab.ravel(), wro.astype(BF16).ravel()])
    blob2 = np.concatenate([wtab8.ravel(), w18.ravel(), w28.ravel()])
    assert blob1.shape[0] == B1_N and blob2.shape[0] == B2_N

    fbp = np.zeros((128, 7), F32)
    fbp[0:32, 0] = conv_bias
    fbp[:, 1:3] = mbs[0].reshape(2, 128).T
    fbp[:, 3] = mbs[1]
    fbp[0:32, 4] = mbs[2]
    fbp[0:8, 5] = mbs[3]
    fbp[0, 6] = mbs[4][0]

    # ---- per-core data ----
    types = (ef[:, 0] * 64 + ef[:, 1] * 8 + ef[:, 2]).astype(np.int64)
    counts_all = np.zeros((NCORES, NTYPES), np.int64)
    for c in range(NCORES):
        counts_all[c] = np.bincount(types[c * EPC:(c + 1) * EPC], minlength=NTYPES)
    C = max(64, int(np.ceil(counts_all.max() / 64)) * 64)
    assert counts_all.min(axis=1).max() < C  # every core has a padded slot

    in_maps = []
    for c in range(NCORES):
        nsl = slice(c * NPC, (c + 1) * NPC)
        esl = slice(c * EPC, (c + 1) * EPC)
        nf_c = nf[nsl]
        t_c = types[esl]
        src_c = src[esl] - c * NPC
        dst_c = dst[esl] - c * NPC
        cnt = counts_all[c]

        # encoder gather indices, feature-column major
        eidx = (np.arange(9)[:, None] * 128 + nf_c.T).reshape(-1)   # [9*5120]

        # type-sort: edge e -> column t*C + rank
        order = np.argsort(t_c, kind="stable")
        rank = np.empty(EPC, np.int64)
        off = np.concatenate([[0], np.cumsum(cnt)[:-1]])
        rank[order] = np.arange(EPC) - off[t_c[order]]
        pos = t_c * C + rank                                        # [EPC]
        xs_idx = np.full(NTYPES * C, NPC, np.int64)                 # pad -> zero row
        xs_idx[pos] = src_c
        xt_i = np.concatenate([xs_idx, np.arange(NPC)])

        # regather: graph-order 128-row tiles (80 real + 48 pad)
        tmin = int(np.argmin(cnt))
        zslot = tmin * C + int(cnt[tmin])
        rg = np.full((GPC, 128), zslot, np.int64)
        rg[:, :80] = pos.reshape(GPC, 80)
        rg_i = rg.reshape(-1)

        idx16 = np.concatenate(
            [_wrap16(eidx), _wrap16(xt_i), _wrap16(rg_i)], axis=1)

        # compact dst: slot k (partition) x graph g -> dst node in 0..39
        dstc = np.full((128, GPC), -1, np.int16)
        dstc[:80, :] = (dst_c - np.repeat(np.arange(GPC), EPG) * NPG) \
            .reshape(GPC, EPG).T
        m = dict(fb=fbp, idx16=idx16, dstc=dstc)
        if use_cc:
            m["wshard"] = blob1[c * S1_N:(c + 1) * S1_N]
            m["wshard8"] = blob2[c * S2_N:(c + 1) * S2_N]
        else:
            m["wshard"] = blob1
            m["wshard8"] = blob2
        in_maps.append(m)
    return in_maps, C, inv_s


def kernel(node_features, edge_features, edge_index, batch,
           atom_emb, bond_emb, gW1, gW2, gW3, root, conv_bias,
           mW1, mb1, mW2, mb2, mW3, mb3, mW4, mb4, mW5, mb5):
    in_maps, C, inv_s = _host_prep(
        node_features, edge_features, edge_index, batch, atom_emb, bond_emb,
        gW1, gW2, gW3, root, conv_bias,
        [mW1, mW2, mW3, mW4, mW5], [mb1, mb2, mb3, mb4, mb5])
    nc = _build_program(C, inv_s)
    res = run_bass_kernel_spmd(nc, in_maps, list(range(NCORES)))
    y = np.concatenate([r["y"].reshape(GPC) for r in res.results])
    return y.reshape(G, 1).astype(F32)
